# revision 16
# baseline (speedup 1.0000x reference)
"""CGC layer (MoE routing) kernel for 8 Trainium2 NeuronCores.

Strategy: data-parallel over the batch (8192 / 8 = 1024 rows per core,
params replicated, no collectives).

Per core, for each of 16 experts (4 shared + 12 task-specific) and each
128-row batch tile (a flattened 128-block pipeline):
  z[b,h] (PSUM, fp32) = bias prefill (ScalarE copy / DVE tensor_copy,
                        emitted 2 blocks ahead so it never gates the PE
                        stream) + 16 accumulating bf16 matmuls (8 K-chunks
                        x 2 PSUM-bank halves, x-transpose chunk stationary,
                        halves alternate banks so consecutive MMs pipeline)
  specific experts (1 consumer): ScalarE rg = Relu(z * gate)  (gate folded
                        into the activation's per-partition scale), then
                        acc[t] += rg  (plain tensor add on the Pool engine)
  shared experts (3 consumers): ScalarE r = Relu(z), then per task
                        acc[t] (+)= gate * r  (DVE tensor_scalar /
                        scalar_tensor_tensor; Pool rejects AP-scalar ops)
The PE matmul stream is the critical path; gate logits share the 3
rotating 2-bank z slots rather than their own PSUM pool (6 banks total).

Gates: one bf16 GEMM (K=1024, N=24) per batch tile + softmax per task
group of 8 (Exp on ScalarE, sum/reciprocal/scale on VectorE), all
up-front; the z-slot has_written warmup re-runs after them because
their start=True marks whole PSUM zero-regions pending.

Inputs are sharded + transposed + cast to bf16 on the host; output is
gathered on the host. All shapes hardcoded for the problem instance.
"""

import numpy as np
import ml_dtypes

import concourse.mybir as mybir
import concourse.tile as tile
from concourse import bacc
from concourse.bass_utils import run_bass_kernel_spmd

D = 1024          # d_model
H = 1024          # expert_dim
T = 3             # tasks
NSHARED = 4
NSPEC = 4
NE = NSHARED + T * NSPEC    # 16 experts total (shared first)
NG = NSPEC + NSHARED        # 8 gate candidates per task
B = 8192
N_CORES = 8
BL = B // N_CORES           # 1024 rows per core
P = 128                     # partitions
KC = D // P                 # 8 K-chunks
NT = BL // P                # 8 batch tiles per core
NH = H // 512               # 2 PSUM half-tiles

F32 = mybir.dt.float32
BF16 = mybir.dt.bfloat16
ACT = mybir.ActivationFunctionType
ALU = mybir.AluOpType
AXIS = mybir.AxisListType

BF16_NP = ml_dtypes.bfloat16


def _consumers(e):
    """Expert index -> list of (task, gate column in the 24-wide layout)."""
    if e < NSHARED:
        return [(t, t * NG + NSPEC + e) for t in range(T)]
    t, j = divmod(e - NSHARED, NSPEC)
    return [(t, t * NG + j)]


def _build_nc(repeat=1):
    """repeat>1 re-runs the whole compute body (timing builds only)."""
    nc = bacc.Bacc(None, target_bir_lowering=False)

    xT_d = nc.dram_tensor("xT", (D, BL), BF16, kind="ExternalInput")
    w_d = nc.dram_tensor("W", (NE, D, H), BF16, kind="ExternalInput")
    b_d = nc.dram_tensor("bias", (NE, P, H), BF16, kind="ExternalInput")
    wg_d = nc.dram_tensor("Wg", (P, KC, T * NG), BF16, kind="ExternalInput")
    out_d = nc.dram_tensor("out", (T, BL, H), F32, kind="ExternalOutput")

    with tile.TileContext(nc) as tc:
        with (
            tc.tile_pool(name="xp", bufs=1) as xp,
            tc.tile_pool(name="wp", bufs=2) as wp,
            tc.tile_pool(name="bp", bufs=2) as bp,
            tc.tile_pool(name="cp", bufs=1) as cp,
            tc.tile_pool(name="gp", bufs=1) as gp,
            tc.tile_pool(name="rp", bufs=3) as rp,
            tc.tile_pool(name="accp", bufs=1) as accp,
            tc.tile_pool(name="ps", bufs=4, space="PSUM") as ps,
        ):
            # ---- constants / activations in SBUF ----
            # xT split into per-chunk DMAs so the first expert's matmuls can
            # start as soon as the early chunks land.
            xT = xp.tile([P, KC, BL], BF16)
            for c in range(KC):
                nc.sync.dma_start(xT[:, c, :], xT_d[c * P:(c + 1) * P, :])
            wg = cp.tile([P, KC, T * NG], BF16, tag="wg")
            nc.sync.dma_start(wg[:], wg_d[:])
            ones = cp.tile([1, P], BF16, tag="ones")
            nc.vector.memset(ones[:], 1.0)

            zrow = cp.tile([1, 512], BF16, tag="zrow")
            nc.vector.memset(zrow[:], 0.0)

            def warmup_slots():
                # Set the has_written bits of every byte of the 4 z slots so
                # accumulating (start=False) matmuls on top of the engine
                # bias prefill ADD instead of overwriting.  Must re-run after
                # the gate matmuls: their start=True marks the whole PSUM
                # zero-region pending, which would make the first expert
                # matmul drop the prefilled bias outside the gate columns.
                for _slot in range(4):
                    zi = ps.tile([P, H], F32, tag="z")
                    for n in range(NH):
                        nc.tensor.matmul(
                            zi[:, n * 512:(n + 1) * 512], ones[:], zrow[:],
                            start=True, stop=True,
                        )

            # run once up-front as well: these matmuls have no DMA
            # dependencies, so they fill the PE idle window while the first
            # xT/W chunks land and keep the HAM clock-gate warm
            warmup_slots()

            def emit_body():
                # ---- gates for every batch tile ----
                gates = []
                for i in range(NT):
                    # gate psum lives in the z pool (first 24 cols of a
                    # full slot) instead of a dedicated PSUM pool
                    pgf = ps.tile([P, H], F32, tag="z")
                    pg = pgf[:, :T * NG]
                    for c in range(KC):
                        nc.tensor.matmul(
                            pg,
                            xT[:, c, i * P:(i + 1) * P],
                            wg[:, c, :],
                            start=(c == 0),
                            stop=(c == KC - 1),
                        )
                    ex = gp.tile([P, T * NG], F32, tag=f"ex{i}")
                    nc.scalar.activation(ex[:], pg, ACT.Exp)
                    s = gp.tile([P, T], F32, tag=f"gs{i}")
                    for t in range(T):
                        nc.vector.tensor_reduce(
                            s[:, t:t + 1], ex[:, t * NG:(t + 1) * NG],
                            axis=AXIS.X, op=ALU.add,
                        )
                    rcp = gp.tile([P, T], F32, tag=f"gr{i}")
                    nc.vector.reciprocal(rcp[:], s[:])
                    g = gp.tile([P, T * NG], F32, tag=f"g{i}")
                    for t in range(T):
                        nc.vector.tensor_scalar(
                            g[:, t * NG:(t + 1) * NG],
                            ex[:, t * NG:(t + 1) * NG],
                            rcp[:, t:t + 1], None, op0=ALU.mult,
                        )
                    gates.append(g)
                warmup_slots()

                # ---- expert loop (shared experts first), flattened into a
                # 128-block (expert, tile) pipeline.  The PSUM bias prefill
                # for block k+2 is emitted right after block k-1's relu (the
                # op that frees its PSUM slot), so in each engine's FIFO the
                # prefill is never queued behind work that runs only after
                # the matmuls it gates — the PE stream never waits.
                acc = {}
                w_tiles = {}
                be_tiles = {}
                seq = [(e, i) for e in range(NE) for i in range(NT)]
                AHEAD = 3                  # matches ps bufs=4 rotation
                z_of = {}

                def ensure_w(e):
                    if e in w_tiles:
                        return
                    w = wp.tile([P, KC, H], BF16)
                    w_tiles[e] = w
                    # per-chunk DMAs: lets the first expert start before its
                    # full 2MB lands, and spreads the DMA bursts of later
                    # experts so they interfere less with the PE stream
                    for c in range(KC):
                        nc.sync.dma_start(
                            w[:, c, :], w_d[e][c * P:(c + 1) * P, :])

                def prefill(k):
                    e, i = seq[k]
                    if e not in be_tiles:
                        be = bp.tile([P, H], BF16)
                        be_tiles[e] = be
                        nc.sync.dma_start(be[:], b_d[e])
                    z = ps.tile([P, H], F32, tag="z")
                    z_of[k] = z
                    if k % 8 < 5:
                        nc.scalar.copy(z[:], be_tiles[e][:])
                    else:
                        nc.vector.tensor_copy(z[:], be_tiles[e][:])

                for k, (e, i) in enumerate(seq):
                    if i == 0:
                        ensure_w(e)
                    if k == 0:
                        for kk in range(AHEAD + 1):
                            prefill(kk)
                    elif k + AHEAD < len(seq):
                        prefill(k + AHEAD)
                    z = z_of.pop(k)
                    w = w_tiles[e]
                    for c in range(KC):
                        lhsT = xT[:, c, i * P:(i + 1) * P]
                        for n in range(NH):
                            nc.tensor.matmul(
                                z[:, n * 512:(n + 1) * 512],
                                lhsT,
                                w[:, c, n * 512:(n + 1) * 512],
                                start=False, stop=(c == KC - 1),
                                skip_group_check=True,
                            )
                    if e < NSHARED:
                        # 3 consumers: plain relu, then gated MACs.  Pool
                        # rejects AP-scalar ops (TensorScalarPtr), so the
                        # gate-scalar MACs stay on DVE.
                        r = rp.tile([P, H], F32)
                        nc.scalar.activation(r[:], z[:], ACT.Relu)
                        for (t, col) in _consumers(e):
                            gcol = gates[i][:, col:col + 1]
                            if (t, i) not in acc:
                                a = accp.tile([P, H], F32, tag=f"acc{t}_{i}")
                                acc[(t, i)] = a
                                nc.vector.tensor_scalar(
                                    a[:], r[:], gcol, None, op0=ALU.mult,
                                )
                            else:
                                a = acc[(t, i)]
                                nc.vector.scalar_tensor_tensor(
                                    a[:], r[:], gcol, a[:],
                                    op0=ALU.mult, op1=ALU.add,
                                )
                    else:
                        # 1 consumer: fold the gate into the relu's
                        # per-partition scale (gate > 0 from softmax); the
                        # plain accumulate add runs on the Pool engine.
                        (t, col) = _consumers(e)[0]
                        gcol = gates[i][:, col:col + 1]
                        rg = rp.tile([P, H], F32)
                        nc.scalar.activation(
                            rg[:], z[:], ACT.Relu, scale=gcol)
                        a = acc[(t, i)]
                        nc.gpsimd.tensor_add(a[:], a[:], rg[:])
                    # task t is complete once its last specific expert's
                    # contribution for this batch tile has landed
                    if e >= NSHARED and (e - NSHARED) % NSPEC == NSPEC - 1:
                        t_done = (e - NSHARED) // NSPEC
                        nc.sync.dma_start(
                            out_d[t_done, i * P:(i + 1) * P, :],
                            acc[(t_done, i)][:],
                        )
                # allow repeat>1 timing builds to re-enter cleanly
                w_tiles.clear()
                be_tiles.clear()

            for _ in range(repeat):
                emit_body()

    nc.compile()
    return nc


_NC_CACHE = None


def _get_nc():
    global _NC_CACHE
    if _NC_CACHE is None:
        _NC_CACHE = _build_nc()
    return _NC_CACHE


def prep_inputs(x, Ws, bs, Wt, bt, Wg):
    """Host-side shard/cast/transpose: returns per-core input maps."""
    x = np.asarray(x)
    # expert order: shared(4) then task-specific t-major (12)
    w_all = np.concatenate(
        [np.asarray(Ws), np.asarray(Wt).reshape(T * NSPEC, D, H)], axis=0
    ).astype(BF16_NP)                                  # (16, D, H)
    b_all = np.concatenate(
        [np.asarray(bs), np.asarray(bt).reshape(T * NSPEC, H)], axis=0
    ).astype(BF16_NP)                                  # (16, H)
    b_all = np.ascontiguousarray(
        np.broadcast_to(b_all[:, None, :], (NE, P, H)))    # (16, 128, H)
    # reference gate candidate order is [specific(4), shared(4)]; our
    # gate column layout is t*8 + [0..3]=specific j, [4..7]=shared s.
    wg_all = np.ascontiguousarray(
        np.asarray(Wg).transpose(1, 0, 2).reshape(KC, P, T * NG).transpose(1, 0, 2)
    ).astype(BF16_NP)                                  # (P, KC, 24), contiguous rows

    in_maps = []
    for c in range(N_CORES):
        xs = x[c * BL:(c + 1) * BL]                    # (BL, D)
        xT = np.ascontiguousarray(xs.T).astype(BF16_NP)  # (D, BL)
        in_maps.append({"xT": xT, "W": w_all, "bias": b_all, "Wg": wg_all})
    return in_maps


def kernel(x, Ws, bs, Wt, bt, Wg):
    """Full-input entry point: shard, run on 8 cores, gather."""
    in_maps = prep_inputs(x, Ws, bs, Wt, bt, Wg)
    nc = _get_nc()
    res = run_bass_kernel_spmd(nc, in_maps, core_ids=list(range(N_CORES)))
    out = np.concatenate([res.results[c]["out"] for c in range(N_CORES)], axis=1)
    return out


# revision 17
# speedup vs baseline: 1.1081x; 1.1081x over previous
"""CGC layer (MoE routing) kernel for 8 Trainium2 NeuronCores.

Strategy: data-parallel over the batch (8192 / 8 = 1024 rows per core,
params replicated, no collectives).

Per core, for each of 16 experts (4 shared + 12 task-specific) and each
128-row batch tile (a flattened 128-block pipeline):
  z[b,h] (PSUM, fp32) = bias prefill (ScalarE copy / DVE tensor_copy,
                        emitted 2 blocks ahead so it never gates the PE
                        stream) + 16 accumulating bf16 matmuls (8 K-chunks
                        x 2 PSUM-bank halves, x-transpose chunk stationary,
                        halves alternate banks so consecutive MMs pipeline)
  specific experts (1 consumer): ScalarE rg = Relu(z * gate)  (gate folded
                        into the activation's per-partition scale), then
                        acc[t] += rg  (plain tensor add on the Pool engine)
  shared experts (3 consumers): ScalarE r = Relu(z), then per task
                        acc[t] (+)= gate * r  (DVE tensor_scalar /
                        scalar_tensor_tensor; Pool rejects AP-scalar ops)
The PE matmul stream is the critical path; gate logits share the 3
rotating 2-bank z slots rather than their own PSUM pool (6 banks total).

Gates: one bf16 GEMM (K=1024, N=24) per batch tile + softmax per task
group of 8 (Exp on ScalarE, sum/reciprocal/scale on VectorE), all
up-front; the z-slot has_written warmup re-runs after them because
their start=True marks whole PSUM zero-regions pending.

Inputs are sharded + transposed + cast to bf16 on the host; output is
gathered on the host. All shapes hardcoded for the problem instance.
"""

import numpy as np
import ml_dtypes

import concourse.mybir as mybir
import concourse.tile as tile
from concourse import bacc
from concourse.bass_utils import run_bass_kernel_spmd

D = 1024          # d_model
H = 1024          # expert_dim
T = 3             # tasks
NSHARED = 4
NSPEC = 4
NE = NSHARED + T * NSPEC    # 16 experts total (shared first)
NG = NSPEC + NSHARED        # 8 gate candidates per task
B = 8192
N_CORES = 8
BL = B // N_CORES           # 1024 rows per core
P = 128                     # partitions
KC = D // P                 # 8 K-chunks
NT = BL // P                # 8 batch tiles per core
NH = H // 512               # 2 PSUM half-tiles

F32 = mybir.dt.float32
BF16 = mybir.dt.bfloat16
ACT = mybir.ActivationFunctionType
ALU = mybir.AluOpType
AXIS = mybir.AxisListType

BF16_NP = ml_dtypes.bfloat16


def _consumers(e):
    """Expert index -> list of (task, gate column in the 24-wide layout)."""
    if e < NSHARED:
        return [(t, t * NG + NSPEC + e) for t in range(T)]
    t, j = divmod(e - NSHARED, NSPEC)
    return [(t, t * NG + j)]


def _build_nc(repeat=1):
    """repeat>1 re-runs the whole compute body (timing builds only)."""
    nc = bacc.Bacc(None, target_bir_lowering=False)

    xT_d = nc.dram_tensor("xT", (D, BL), BF16, kind="ExternalInput")
    w_d = nc.dram_tensor("W", (NE, D, H), BF16, kind="ExternalInput")
    b_d = nc.dram_tensor("bias", (NE, P, H), BF16, kind="ExternalInput")
    wg_d = nc.dram_tensor("Wg", (P, KC, T * NG), BF16, kind="ExternalInput")
    out_d = nc.dram_tensor("out", (T, BL, H), F32, kind="ExternalOutput")

    with tile.TileContext(nc) as tc:
        with (
            tc.tile_pool(name="xp", bufs=1) as xp,
            tc.tile_pool(name="wp", bufs=3) as wp,
            tc.tile_pool(name="bp", bufs=3) as bp,
            tc.tile_pool(name="cp", bufs=1) as cp,
            tc.tile_pool(name="gp", bufs=1) as gp,
            tc.tile_pool(name="rp", bufs=4) as rp,
            tc.tile_pool(name="accp", bufs=1) as accp,
            tc.tile_pool(name="ps", bufs=4, space="PSUM") as ps,
        ):
            # ---- constants / activations in SBUF ----
            # xT split into per-chunk DMAs so the first expert's matmuls can
            # start as soon as the early chunks land.
            xT = xp.tile([P, KC, BL], BF16)
            for c in range(KC):
                nc.sync.dma_start(xT[:, c, :], xT_d[c * P:(c + 1) * P, :])
            wg = cp.tile([P, KC, T * NG], BF16, tag="wg")
            nc.sync.dma_start(wg[:], wg_d[:])
            ones = cp.tile([1, P], BF16, tag="ones")
            nc.vector.memset(ones[:], 1.0)

            zrow = cp.tile([1, 512], BF16, tag="zrow")
            nc.vector.memset(zrow[:], 0.0)

            def warmup_slots():
                # Set the has_written bits of every byte of the 4 z slots so
                # accumulating (start=False) matmuls on top of the engine
                # bias prefill ADD instead of overwriting.  Must re-run after
                # the gate matmuls: their start=True marks the whole PSUM
                # zero-region pending, which would make the first expert
                # matmul drop the prefilled bias outside the gate columns.
                for _slot in range(4):
                    zi = ps.tile([P, H], F32, tag="z")
                    for n in range(NH):
                        nc.tensor.matmul(
                            zi[:, n * 512:(n + 1) * 512], ones[:], zrow[:],
                            start=True, stop=True,
                        )

            # run once up-front as well: these matmuls have no DMA
            # dependencies, so they fill the PE idle window while the first
            # xT/W chunks land and keep the HAM clock-gate warm
            warmup_slots()

            def emit_body():
                # ---- gates for every batch tile ----
                gates = []
                for i in range(NT):
                    # gate psum lives in the z pool (first 24 cols of a
                    # full slot) instead of a dedicated PSUM pool
                    pgf = ps.tile([P, H], F32, tag="z")
                    pg = pgf[:, :T * NG]
                    for c in range(KC):
                        nc.tensor.matmul(
                            pg,
                            xT[:, c, i * P:(i + 1) * P],
                            wg[:, c, :],
                            start=(c == 0),
                            stop=(c == KC - 1),
                        )
                    ex = gp.tile([P, T * NG], F32, tag=f"ex{i}")
                    nc.scalar.activation(ex[:], pg, ACT.Exp)
                    s = gp.tile([P, T], F32, tag=f"gs{i}")
                    for t in range(T):
                        nc.vector.tensor_reduce(
                            s[:, t:t + 1], ex[:, t * NG:(t + 1) * NG],
                            axis=AXIS.X, op=ALU.add,
                        )
                    rcp = gp.tile([P, T], F32, tag=f"gr{i}")
                    nc.vector.reciprocal(rcp[:], s[:])
                    g = gp.tile([P, T * NG], F32, tag=f"g{i}")
                    for t in range(T):
                        nc.vector.tensor_scalar(
                            g[:, t * NG:(t + 1) * NG],
                            ex[:, t * NG:(t + 1) * NG],
                            rcp[:, t:t + 1], None, op0=ALU.mult,
                        )
                    gates.append(g)
                warmup_slots()

                # ---- expert loop (shared experts first), flattened into a
                # 128-block (expert, tile) pipeline.  The PSUM bias prefill
                # for block k+2 is emitted right after block k-1's relu (the
                # op that frees its PSUM slot), so in each engine's FIFO the
                # prefill is never queued behind work that runs only after
                # the matmuls it gates — the PE stream never waits.
                acc = {}
                w_tiles = {}
                be_tiles = {}
                seq = [(e, i) for e in range(NE) for i in range(NT)]
                AHEAD = 3                  # matches ps bufs=4 rotation
                z_of = {}

                def ensure_w(e):
                    if e in w_tiles:
                        return
                    w = wp.tile([P, KC, H], BF16)
                    w_tiles[e] = w
                    # per-chunk DMAs: lets the first expert start before its
                    # full 2MB lands, and spreads the DMA bursts of later
                    # experts so they interfere less with the PE stream
                    for c in range(KC):
                        nc.sync.dma_start(
                            w[:, c, :], w_d[e][c * P:(c + 1) * P, :])

                def prefill(k):
                    e, i = seq[k]
                    if e not in be_tiles:
                        be = bp.tile([P, H], BF16)
                        be_tiles[e] = be
                        nc.sync.dma_start(be[:], b_d[e])
                    z = ps.tile([P, H], F32, tag="z")
                    z_of[k] = z
                    if k % 8 < 5:
                        nc.scalar.copy(z[:], be_tiles[e][:])
                    else:
                        nc.vector.tensor_copy(z[:], be_tiles[e][:])

                for k, (e, i) in enumerate(seq):
                    if i == 0:
                        ensure_w(e)
                    if k == 0:
                        for kk in range(AHEAD + 1):
                            prefill(kk)
                    elif k + AHEAD < len(seq):
                        prefill(k + AHEAD)
                    z = z_of.pop(k)
                    w = w_tiles[e]
                    for c in range(KC):
                        lhsT = xT[:, c, i * P:(i + 1) * P]
                        for n in range(NH):
                            nc.tensor.matmul(
                                z[:, n * 512:(n + 1) * 512],
                                lhsT,
                                w[:, c, n * 512:(n + 1) * 512],
                                start=False, stop=(c == KC - 1),
                                skip_group_check=True,
                            )
                    if e < NSHARED:
                        # 3 consumers: plain relu, then gated MACs.  Pool
                        # rejects AP-scalar ops (TensorScalarPtr), so the
                        # gate-scalar MACs stay on DVE.
                        r = rp.tile([P, H], F32)
                        nc.scalar.activation(r[:], z[:], ACT.Relu)
                        for (t, col) in _consumers(e):
                            gcol = gates[i][:, col:col + 1]
                            if (t, i) not in acc:
                                a = accp.tile([P, H], F32, tag=f"acc{t}_{i}")
                                acc[(t, i)] = a
                                nc.vector.tensor_scalar(
                                    a[:], r[:], gcol, None, op0=ALU.mult,
                                )
                            else:
                                a = acc[(t, i)]
                                nc.vector.scalar_tensor_tensor(
                                    a[:], r[:], gcol, a[:],
                                    op0=ALU.mult, op1=ALU.add,
                                )
                    else:
                        # 1 consumer: fold the gate into the relu's
                        # per-partition scale (gate > 0 from softmax); the
                        # plain accumulate add runs on the Pool engine.
                        (t, col) = _consumers(e)[0]
                        gcol = gates[i][:, col:col + 1]
                        rg = rp.tile([P, H], F32)
                        nc.scalar.activation(
                            rg[:], z[:], ACT.Relu, scale=gcol)
                        a = acc[(t, i)]
                        nc.gpsimd.tensor_add(a[:], a[:], rg[:])
                    # task t is complete once its last specific expert's
                    # contribution for this batch tile has landed
                    if e >= NSHARED and (e - NSHARED) % NSPEC == NSPEC - 1:
                        t_done = (e - NSHARED) // NSPEC
                        nc.sync.dma_start(
                            out_d[t_done, i * P:(i + 1) * P, :],
                            acc[(t_done, i)][:],
                        )
                # allow repeat>1 timing builds to re-enter cleanly
                w_tiles.clear()
                be_tiles.clear()

            for _ in range(repeat):
                emit_body()

    nc.compile()
    return nc


_NC_CACHE = None


def _get_nc():
    global _NC_CACHE
    if _NC_CACHE is None:
        _NC_CACHE = _build_nc()
    return _NC_CACHE


def prep_inputs(x, Ws, bs, Wt, bt, Wg):
    """Host-side shard/cast/transpose: returns per-core input maps."""
    x = np.asarray(x)
    # expert order: shared(4) then task-specific t-major (12)
    w_all = np.concatenate(
        [np.asarray(Ws), np.asarray(Wt).reshape(T * NSPEC, D, H)], axis=0
    ).astype(BF16_NP)                                  # (16, D, H)
    b_all = np.concatenate(
        [np.asarray(bs), np.asarray(bt).reshape(T * NSPEC, H)], axis=0
    ).astype(BF16_NP)                                  # (16, H)
    b_all = np.ascontiguousarray(
        np.broadcast_to(b_all[:, None, :], (NE, P, H)))    # (16, 128, H)
    # reference gate candidate order is [specific(4), shared(4)]; our
    # gate column layout is t*8 + [0..3]=specific j, [4..7]=shared s.
    wg_all = np.ascontiguousarray(
        np.asarray(Wg).transpose(1, 0, 2).reshape(KC, P, T * NG).transpose(1, 0, 2)
    ).astype(BF16_NP)                                  # (P, KC, 24), contiguous rows

    in_maps = []
    for c in range(N_CORES):
        xs = x[c * BL:(c + 1) * BL]                    # (BL, D)
        xT = np.ascontiguousarray(xs.T).astype(BF16_NP)  # (D, BL)
        in_maps.append({"xT": xT, "W": w_all, "bias": b_all, "Wg": wg_all})
    return in_maps


def kernel(x, Ws, bs, Wt, bt, Wg):
    """Full-input entry point: shard, run on 8 cores, gather."""
    in_maps = prep_inputs(x, Ws, bs, Wt, bt, Wg)
    nc = _get_nc()
    res = run_bass_kernel_spmd(nc, in_maps, core_ids=list(range(N_CORES)))
    out = np.concatenate([res.results[c]["out"] for c in range(N_CORES)], axis=1)
    return out
